# revision 25
# baseline (speedup 1.0000x reference)
"""Multi-head dot-product attention with prefix KV, on 8 trn2 NeuronCores.

Sharding: batch (2) x head-groups (4 groups of 4 heads) = 8 cores.
Each core computes q/k/v projections for its 4 heads, flash-style
attention (scores kept transposed: [kv, L] so no on-device transposes
are needed), and a partial out-projection [E, L]; the host sums the 4
head-group partials per batch and transposes back.

Device-side layout notes:
  - Host pre-transposes inputs_q/inputs_kv to x^T [E, L] so both the
    qT/kT projections (lhsT=W) and the natural-layout v projection
    (lhsT=x^T tiles) need no on-device transposes.
  - kv axis is padded to 2176 = 17*128: chunk 0 = [prefix(64) | dead(64)],
    chunks 1..16 = kv positions.  Dead columns are killed with a
    per-partition -1e10 bias on the chunk-0 exp.
  - softmax runs without max subtraction (scores are O(1); masked
    entries underflow to exactly 0 like the reference's -1e10 bias).
  - denominator comes free as an extra ones-column in the v weights
    (M=65 ctx matmul); normalization uses a K=1 outer-product matmul to
    broadcast 1/denom across partitions.
"""

import numpy as np

B, LQ, LKV, E, H, D, P = 2, 2048, 2048, 1024, 16, 64, 64
NCORES = 8
HGROUPS = 4          # head groups (cores per batch)
HPC = H // HGROUPS   # heads per core = 4
KVPAD = 128 + LKV    # 2176
NCH = KVPAD // 128   # 17 chunks
NG = LQ // 512       # 4 L-groups of 512
NEG = -1.0e10

_CACHE = {}


def _build_module(plan, debug_taps=False):
    """Build the single-core Bass module (same program for all 8 cores)."""
    import concourse.bass as bass
    import concourse.tile as tile
    import concourse.mybir as mybir
    from concourse import bacc

    f32 = mybir.dt.float32
    f32r = mybir.dt.float32r
    Exp = mybir.ActivationFunctionType.Exp
    Log = mybir.ActivationFunctionType.Ln

    chunks, mixed_idx, nmix = plan["chunks"], plan["mixed_idx"], plan["nmix"]

    nc = bacc.Bacc("TRN2", target_bir_lowering=False, debug=False,
                   enable_asserts=False, num_devices=NCORES)

    xqT_d = nc.dram_tensor("xqT", [E, LQ], f32r, kind="ExternalInput").ap()
    xkvT_d = nc.dram_tensor("xkvT", [E, LKV], f32r, kind="ExternalInput").ap()
    wq_d = nc.dram_tensor("wq", [E, HPC * D], f32r, kind="ExternalInput").ap()
    wk_d = nc.dram_tensor("wk", [E, HPC * D], f32r, kind="ExternalInput").ap()
    wv_d = nc.dram_tensor("wv", [E, HPC * D], f32r, kind="ExternalInput").ap()
    wo_d = nc.dram_tensor("wo", [HPC * D, E], f32r, kind="ExternalInput").ap()
    kprefT_d = nc.dram_tensor("kprefT", [2, 128, 128], f32r, kind="ExternalInput").ap()
    vpref_d = nc.dram_tensor("vpref", [128, HPC, D], f32r, kind="ExternalInput").ap()
    vones_d = nc.dram_tensor("vones", [128, NCH, HPC], f32r, kind="ExternalInput").ap()
    onescol_d = nc.dram_tensor("onescol", [1, 64], f32r, kind="ExternalInput").ap()
    if nmix:
        maskblk_d = nc.dram_tensor("maskblk", [nmix, 128, 512], f32,
                                   kind="ExternalInput").ap()
    outT_d = nc.dram_tensor("outT", [E, LQ], f32, kind="ExternalOutput").ap()
    if debug_taps:
        dbg = {
            "dbg_QT": nc.dram_tensor("dbg_QT", [2, 128, LQ], f32r, kind="ExternalOutput").ap(),
            "dbg_KT": nc.dram_tensor("dbg_KT", [2, 128, KVPAD], f32r, kind="ExternalOutput").ap(),
            "dbg_VT": nc.dram_tensor("dbg_VT", [128, NCH, HPC, 65], f32r, kind="ExternalOutput").ap(),
            "dbg_CTXT": nc.dram_tensor("dbg_CTXT", [2, 128, LQ], f32r, kind="ExternalOutput").ap(),
        }

    from contextlib import ExitStack
    with tile.TileContext(nc) as tc, ExitStack() as stk:
        # ---- persistent SBUF tiles (one pool, one tag per tile) ----
        pers = stk.enter_context(tc.tile_pool(name="pers", bufs=1))

        def ptile(shape, name, dt=None):
            return pers.tile(shape, dt or f32r, tag=name, name=name)

        wq_sb = ptile([128, 8, 256], "wq_sb")
        wk_sb = ptile([128, 8, 256], "wk_sb")
        wv_sb = ptile([128, 8, 256], "wv_sb")
        wo_sb = ptile([128, 2, 1024], "wo_sb")
        QT = [ptile([128, LQ], f"QT{i}") for i in range(2)]
        KT2 = [ptile([128, KVPAD], f"KT{i}") for i in range(2)]
        VT = ptile([128, NCH, HPC, 65], "VT")
        CTXT = [[ptile([128, 512], f"CTXT{i}g{g}") for g in range(NG)]
                for i in range(2)]
        cb0 = ptile([128, 1], "cb0", f32)
        ones_col = ptile([128, 64], "ones_col")

        nc.sync.dma_start(out=wq_sb, in_=wq_d.rearrange("(ec p) m -> p ec m", p=128))
        nc.sync.dma_start(out=wk_sb, in_=wk_d.rearrange("(ec p) m -> p ec m", p=128))
        nc.sync.dma_start(out=wv_sb, in_=wv_d.rearrange("(ec p) m -> p ec m", p=128))
        nc.sync.dma_start(out=wo_sb, in_=wo_d.rearrange("(hc p) e -> p hc e", p=128))

        # init via DMA'd host constants (memset cannot write f32r):
        # ones columns of VT, chunk-0 [prefix|dead-zero] planes, chunk0 bias.
        nc.vector.memset(cb0[0:64, :], 0.0)
        nc.vector.memset(cb0[64:128, :], NEG)
        nc.sync.dma_start(out=VT[:, :, :, 64:65], in_=vones_d)
        nc.sync.dma_start(out=ones_col[64:65, :], in_=onescol_d)
        for hc in range(2):
            nc.sync.dma_start(out=KT2[hc][:, 0:128], in_=kprefT_d[hc])
        nc.sync.dma_start(out=VT[:, 0, :, 0:D], in_=vpref_d)

        # ---- projections ----
        with tc.tile_pool(name="xio", bufs=2) as xio, \
             tc.tile_pool(name="pp", bufs=1, space="PSUM") as pp:
            for ls in range(8):          # 256-wide L slices
                l0 = 256 * ls
                xq_t = xio.tile([128, 8, 256], f32r, tag="xq", bufs=2)
                xkv_t = xio.tile([128, 8, 256], f32r, tag="xkv", bufs=2)
                nc.sync.dma_start(
                    out=xq_t,
                    in_=xqT_d.rearrange("(ec p) l -> p ec l", p=128)[:, :, l0:l0 + 256])
                nc.sync.dma_start(
                    out=xkv_t,
                    in_=xkvT_d.rearrange("(ec p) l -> p ec l", p=128)[:, :, l0:l0 + 256])
                for t in range(2):
                    ps_q = pp.tile([128, 256], f32, tag="qk", bufs=4)
                    for ec in range(8):
                        nc.tensor.matmul(
                            ps_q,
                            lhsT=wq_sb[:, ec, 128 * t:128 * t + 128],
                            rhs=xq_t[:, ec, :],
                            start=(ec == 0), stop=(ec == 7))
                    nc.vector.tensor_copy(out=QT[t][:, l0:l0 + 256], in_=ps_q)
                    ps_k = pp.tile([128, 256], f32, tag="qk", bufs=4)
                    for ec in range(8):
                        nc.tensor.matmul(
                            ps_k,
                            lhsT=wk_sb[:, ec, 128 * t:128 * t + 128],
                            rhs=xkv_t[:, ec, :],
                            start=(ec == 0), stop=(ec == 7))
                    nc.vector.tensor_copy(
                        out=KT2[t][:, 128 + l0:128 + l0 + 256], in_=ps_k)
                for sub in range(2):
                    ps_v = pp.tile([128, 256], f32, tag="v", bufs=2)
                    for ec in range(8):
                        nc.tensor.matmul(
                            ps_v,
                            lhsT=xkv_t[:, ec, 128 * sub:128 * sub + 128],
                            rhs=wv_sb[:, ec, :],
                            start=(ec == 0), stop=(ec == 7))
                    nc.vector.tensor_copy(
                        out=VT[:, 1 + 2 * ls + sub, :, 0:D],
                        in_=ps_v.rearrange("p (h d) -> p h d", h=HPC))

        # ---- attention ----
        # Head pairs (even head on PE rows 0-63, odd head on rows 64-127)
        # are interleaved so LDWEIGHTS overlaps the other row-group's
        # matmul and the two K=64 score matmuls run concurrently.  The
        # out-projection for each finished L-group is emitted inline to
        # give the PE independent filler work between dependency stalls.
        def mask_mul(dst, a, b):
            nc.vector.tensor_mul(dst, a, b)

        with tc.tile_pool(name="att_ps", bufs=1, space="PSUM") as attps, \
             tc.tile_pool(name="att_sb", bufs=1) as attsb:
            # preload every mixed-block mask tile up front
            mts = {}
            for g in range(NG):
                for c in chunks[g]:
                    if (g, c) in mixed_idx:
                        mt = attsb.tile([128, 512], f32, tag="mask", bufs=max(nmix, 1),
                                        name=f"mt{g}_{c}")
                        nc.sync.dma_start(out=mt, in_=maskblk_d[mixed_idx[(g, c)]])
                        mts[(g, c)] = mt
            for g in range(NG):
                gl = 512 * g
                cs = chunks[g]
                batches = [[cs[0]]] + [cs[1 + i:3 + i] for i in range(0, len(cs) - 1, 2)]
                for hp in range(HPC // 2):
                    heads = (2 * hp, 2 * hp + 1)
                    ctx_ps = {}
                    for h in heads:
                        ctx_ps[h] = attps.tile([65, 512], f32, tag="ctx", bufs=4,
                                               name=f"ctx{h}")
                    sc = {}
                    pr = {}
                    for bi, batch in enumerate(batches):
                        w = 512 * len(batch)
                        for h in heads:
                            sc[h] = attps.tile([128, 1024], f32, tag="sc", bufs=2,
                                               name=f"sc{h}")
                        for j, c in enumerate(batch):
                            for h in heads:
                                prow = 64 * (h % 2)
                                nc.tensor.matmul(
                                    sc[h][:, 512 * j:512 * j + 512],
                                    lhsT=KT2[hp][prow:prow + 64, 128 * c:128 * c + 128],
                                    rhs=QT[hp][prow:prow + 64, gl:gl + 512],
                                    start=True, stop=True)
                        for h in heads:
                            pr[h] = attsb.tile([128, 1024], f32r, tag="probs",
                                               bufs=4, name=f"pr{h}")
                            if batch[0] == 0:
                                nc.scalar.activation(pr[h][:, 0:w], sc[h][:, 0:w],
                                                     Exp, bias=cb0[:, 0:1])
                            else:
                                nc.scalar.activation(pr[h][:, 0:w], sc[h][:, 0:w], Exp)
                        for j, c in enumerate(batch):
                            if (g, c) in mts:
                                for h in heads:
                                    mask_mul(pr[h][:, 512 * j:512 * j + 512],
                                             pr[h][:, 512 * j:512 * j + 512],
                                             mts[(g, c)])
                        for j, c in enumerate(batch):
                            for h in heads:
                                nc.tensor.matmul(
                                    ctx_ps[h],
                                    lhsT=VT[:, c, h % 2 + 2 * hp, :],
                                    rhs=pr[h][:, 512 * j:512 * j + 512],
                                    start=(bi == 0 and j == 0),
                                    stop=(bi == len(batches) - 1 and j == len(batch) - 1))
                    # normalize both heads of the pair
                    for h in heads:
                        par = h % 2
                        # 1/d = exp(-ln d) on ACT: low latency, same table set
                        lnd = attsb.tile([65, 512], f32, tag="lnd", bufs=2)
                        nc.scalar.activation(lnd[64:65, :], ctx_ps[h][64:65, :], Log)
                        rc_t = attsb.tile([65, 512], f32r, tag="recip", bufs=2)
                        nc.scalar.activation(rc_t[64:65, :], lnd[64:65, :], Exp,
                                             scale=-1.0)
                        bc_ps = attps.tile([64, 512], f32, tag="sc", bufs=2)
                        nc.tensor.matmul(bc_ps,
                                         lhsT=ones_col[64:65, :],
                                         rhs=rc_t[64:65, :],
                                         start=True, stop=True)
                        bc = attsb.tile([64, 512], f32, tag="bc", bufs=2)
                        nc.vector.tensor_copy(out=bc, in_=bc_ps)
                        if par == 0:
                            nc.vector.tensor_mul(CTXT[hp][g][0:64, :],
                                                 ctx_ps[h][0:64, :], bc)
                        else:
                            st = attsb.tile([64, 512], f32r, tag="stage", bufs=2)
                            nc.vector.tensor_mul(st, ctx_ps[h][0:64, :], bc)
                            nc.sync.dma_start(out=CTXT[hp][g][64:128, :], in_=st)
                # out-projection for this finished L-group (PE filler work)
                for et in range(8):
                    ops = attps.tile([128, 512], f32, tag="sc", bufs=2, name="ops")
                    for hc in range(2):
                        nc.tensor.matmul(
                            ops,
                            lhsT=wo_sb[:, hc, 128 * et:128 * et + 128],
                            rhs=CTXT[hc][g],
                            start=(hc == 0), stop=(hc == 1))
                    ot = attsb.tile([128, 512], f32, tag="ostage", bufs=3, name="ot")
                    nc.vector.tensor_copy(out=ot, in_=ops)
                    nc.sync.dma_start(
                        out=outT_d[128 * et:128 * et + 128, gl:gl + 512], in_=ot)
    nc.compile()
    return nc


def _make_plan(mask):
    """Block plan from the actual mask (union over batches -> one SPMD plan)."""
    m = np.asarray(mask[:, 0])                       # [B, LQ, LKV] bool
    blk = m.reshape(B, NG, 512, LKV // 128, 128)
    any_b = blk.any(axis=(2, 4)).any(axis=0)         # [NG, 16]
    all_b = blk.all(axis=(2, 4)).all(axis=0)         # [NG, 16]
    chunks, mixed_idx = [], {}
    order = []
    for g in range(NG):
        cl = [0]
        for c in range(1, NCH):
            if any_b[g, c - 1]:
                cl.append(c)
                if not all_b[g, c - 1]:
                    mixed_idx[(g, c)] = len(order)
                    order.append((g, c))
        chunks.append(cl)
    return {"chunks": chunks, "mixed_idx": mixed_idx, "nmix": len(order),
            "order": order}


def _prep_core_inputs(inputs, plan):
    """Per-core input dicts (8 cores: batch-major, then head-group)."""
    inputs_q = np.ascontiguousarray(inputs["inputs_q"], dtype=np.float32)
    inputs_kv = np.ascontiguousarray(inputs["inputs_kv"], dtype=np.float32)
    key_prefix = np.asarray(inputs["key_prefix"], dtype=np.float32)
    value_prefix = np.asarray(inputs["value_prefix"], dtype=np.float32)
    mask = np.asarray(inputs["mask"])
    Wq = np.asarray(inputs["Wq"], dtype=np.float32)
    Wk = np.asarray(inputs["Wk"], dtype=np.float32)
    Wv = np.asarray(inputs["Wv"], dtype=np.float32)
    Wo = np.asarray(inputs["Wo"], dtype=np.float32)

    xT = [np.ascontiguousarray(inputs_q[b].T) for b in range(B)]
    xkT = [np.ascontiguousarray(inputs_kv[b].T) for b in range(B)]

    maskblks = []
    for b in range(B):
        mb = np.empty((max(plan["nmix"], 1), 128, 512), np.float32)
        for i, (g, c) in enumerate(plan["order"]):
            mb[i] = mask[b, 0, 512 * g:512 * g + 512,
                         128 * (c - 1):128 * c].T.astype(np.float32)
        maskblks.append(mb)

    in_maps = []
    for core in range(NCORES):
        b, hg = core // HGROUPS, core % HGROUPS
        hs = slice(HPC * hg, HPC * (hg + 1))
        kpT = key_prefix[b, :, hs, :]                 # [P, 4, D]
        kpT = kpT.transpose(1, 2, 0).reshape(2, 128, P)  # [hc, (2 heads x D), P]
        kpT = np.concatenate(
            [kpT, np.zeros((2, 128, 128 - P), np.float32)], axis=2)
        kpT = np.ascontiguousarray(kpT)
        im = {
            "xqT": xT[b],
            "xkvT": xkT[b],
            "wq": np.ascontiguousarray(
                (Wq[:, hs, :] / np.sqrt(D)).reshape(E, HPC * D).astype(np.float32)),
            "wk": np.ascontiguousarray(Wk[:, hs, :].reshape(E, HPC * D)),
            "wv": np.ascontiguousarray(Wv[:, hs, :].reshape(E, HPC * D)),
            "wo": np.ascontiguousarray(Wo[hs].reshape(HPC * D, E)),
            "kprefT": kpT,
            "vpref": np.ascontiguousarray(np.concatenate(
                [value_prefix[b, :, hs, :],
                 np.zeros((128 - P, HPC, D), np.float32)], axis=0)),
            "vones": np.ones((128, NCH, HPC), np.float32),
            "onescol": np.ones((1, 64), np.float32),
        }
        if plan["nmix"]:
            im["maskblk"] = maskblks[b]
        in_maps.append(im)
    return in_maps


def kernel(**inputs) -> np.ndarray:
    from concourse import bass_utils

    plan = _make_plan(inputs["mask"])
    key = (tuple(tuple(c) for c in plan["chunks"]), tuple(plan["order"]))
    if key not in _CACHE:
        _CACHE[key] = _build_module(plan)
    nc = _CACHE[key]

    in_maps = _prep_core_inputs(inputs, plan)
    res = bass_utils.run_bass_kernel_spmd(nc, in_maps, core_ids=list(range(NCORES)))

    out = np.zeros((B, LQ, E), np.float32)
    for core in range(NCORES):
        b = core // HGROUPS
        out[b] += res.results[core]["outT"].T
    return out


# revision 27
# speedup vs baseline: 1.0054x; 1.0054x over previous
"""Multi-head dot-product attention with prefix KV, on 8 trn2 NeuronCores.

Sharding: batch (2) x head-groups (4 groups of 4 heads) = 8 cores.
Each core computes q/k/v projections for its 4 heads, flash-style
attention (scores kept transposed: [kv, L] so no on-device transposes
are needed), and a partial out-projection [E, L]; the host sums the 4
head-group partials per batch and transposes back.

Device-side layout notes:
  - Host pre-transposes inputs_q/inputs_kv to x^T [E, L] so both the
    qT/kT projections (lhsT=W) and the natural-layout v projection
    (lhsT=x^T tiles) need no on-device transposes.
  - kv axis is padded to 2176 = 17*128: chunk 0 = [prefix(64) | dead(64)],
    chunks 1..16 = kv positions.  Dead columns are killed with a
    per-partition -1e10 bias on the chunk-0 exp.
  - softmax runs without max subtraction (scores are O(1); masked
    entries underflow to exactly 0 like the reference's -1e10 bias).
  - denominator comes free as an extra ones-column in the v weights
    (M=65 ctx matmul); normalization uses a K=1 outer-product matmul to
    broadcast 1/denom across partitions.
"""

import numpy as np

B, LQ, LKV, E, H, D, P = 2, 2048, 2048, 1024, 16, 64, 64
NCORES = 8
HGROUPS = 4          # head groups (cores per batch)
HPC = H // HGROUPS   # heads per core = 4
KVPAD = 128 + LKV    # 2176
NCH = KVPAD // 128   # 17 chunks
NG = LQ // 512       # 4 L-groups of 512
NEG = -1.0e10

_CACHE = {}


def _build_module(plan, debug_taps=False):
    """Build the single-core Bass module (same program for all 8 cores).

    Projections are interleaved with attention L-groups: attention for
    group g only needs kv chunks <= 4g+4 (x^T L-slices <= 2g+1), so the
    projection matmuls for later slices act as dense PE filler while
    attention waits on the softmax (ACT) pipeline.  All big SBUF tensors
    are split into per-slice tiles so Tile's whole-tile dependency
    tracking permits the overlap.
    """
    import concourse.bass as bass
    import concourse.tile as tile
    import concourse.mybir as mybir
    from concourse import bacc
    from contextlib import ExitStack

    f32 = mybir.dt.float32
    f32r = mybir.dt.float32r
    Exp = mybir.ActivationFunctionType.Exp
    Log = mybir.ActivationFunctionType.Ln

    chunks, mixed_idx, nmix = plan["chunks"], plan["mixed_idx"], plan["nmix"]

    nc = bacc.Bacc("TRN2", target_bir_lowering=False, debug=False,
                   enable_asserts=False, num_devices=NCORES)

    xqT_d = nc.dram_tensor("xqT", [E, LQ], f32r, kind="ExternalInput").ap()
    xkvT_d = nc.dram_tensor("xkvT", [E, LKV], f32r, kind="ExternalInput").ap()
    wq_d = nc.dram_tensor("wq", [E, HPC * D], f32r, kind="ExternalInput").ap()
    wk_d = nc.dram_tensor("wk", [E, HPC * D], f32r, kind="ExternalInput").ap()
    wv_d = nc.dram_tensor("wv", [E, HPC * D], f32r, kind="ExternalInput").ap()
    wo_d = nc.dram_tensor("wo", [HPC * D, E], f32r, kind="ExternalInput").ap()
    kprefT_d = nc.dram_tensor("kprefT", [2, 128, 128], f32r, kind="ExternalInput").ap()
    vpref_d = nc.dram_tensor("vpref", [128, HPC, D], f32r, kind="ExternalInput").ap()
    vones_d = nc.dram_tensor("vones", [128, NCH, HPC], f32r, kind="ExternalInput").ap()
    onescol_d = nc.dram_tensor("onescol", [1, 64], f32r, kind="ExternalInput").ap()
    if nmix:
        maskblk_d = nc.dram_tensor("maskblk", [nmix, 128, 512], f32,
                                   kind="ExternalInput").ap()
    outT_d = nc.dram_tensor("outT", [E, LQ], f32, kind="ExternalOutput").ap()

    with tile.TileContext(nc) as tc, ExitStack() as stk:
        pers = stk.enter_context(tc.tile_pool(name="pers", bufs=1))

        def ptile(shape, name, dt=None):
            return pers.tile(shape, dt or f32r, tag=name, name=name)

        wq_sb = ptile([128, 8, 256], "wq_sb")
        wk_sb = ptile([128, 8, 256], "wk_sb")
        wv_sb = ptile([128, 8, 256], "wv_sb")
        wo_sb = ptile([128, 2, 1024], "wo_sb")
        # per-slice tensors: QTS[hc][g] 512-wide; KTS[hc][s]: s=0 prefix
        # [128,128], s>=1 [128,256] (kv chunks 2s-1, 2s); VTS[c] per chunk.
        QTS = [[ptile([128, 512], f"QT{i}g{g}") for g in range(NG)] for i in range(2)]
        KTS = [[ptile([128, 128] if s == 0 else [128, 256], f"KT{i}s{s}")
                for s in range(9)] for i in range(2)]
        VTS = [ptile([128, HPC, 65], f"VT{c}") for c in range(NCH)]
        CTXT = [[ptile([128, 512], f"CTXT{i}g{g}") for g in range(NG)]
                for i in range(2)]
        cb0 = ptile([128, 1], "cb0", f32)
        ones_col = ptile([128, 64], "ones_col")

        def kslice(hc, c):
            if c == 0:
                return KTS[hc][0][:, 0:128]
            s, off = (c + 1) // 2, 128 * ((c - 1) % 2)
            return KTS[hc][s][:, off:off + 128]

        nc.sync.dma_start(out=wq_sb, in_=wq_d.rearrange("(ec p) m -> p ec m", p=128))
        nc.sync.dma_start(out=wk_sb, in_=wk_d.rearrange("(ec p) m -> p ec m", p=128))
        nc.sync.dma_start(out=wv_sb, in_=wv_d.rearrange("(ec p) m -> p ec m", p=128))
        nc.sync.dma_start(out=wo_sb, in_=wo_d.rearrange("(hc p) e -> p hc e", p=128))

        nc.vector.memset(cb0[0:64, :], 0.0)
        nc.vector.memset(cb0[64:128, :], NEG)
        for c in range(NCH):
            nc.sync.dma_start(out=VTS[c][:, :, 64:65], in_=vones_d[:, c, :])
        nc.sync.dma_start(out=ones_col[64:65, :], in_=onescol_d)
        for hc in range(2):
            nc.sync.dma_start(out=KTS[hc][0], in_=kprefT_d[hc])
        nc.sync.dma_start(out=VTS[0][:, :, 0:D], in_=vpref_d)

        xio = stk.enter_context(tc.tile_pool(name="xio", bufs=2))
        pp = stk.enter_context(tc.tile_pool(name="pp", bufs=1, space="PSUM"))
        attps = stk.enter_context(tc.tile_pool(name="att_ps", bufs=1, space="PSUM"))
        attsb = stk.enter_context(tc.tile_pool(name="att_sb", bufs=1))

        def proj_slice(ls):
            """q/k/v projections for one 256-wide L slice."""
            l0 = 256 * ls
            g, half = ls // 2, ls % 2
            xq_t = xio.tile([128, 8, 256], f32r, tag="xq", bufs=2, name="xq_t")
            xkv_t = xio.tile([128, 8, 256], f32r, tag="xkv", bufs=2, name="xkv_t")
            nc.sync.dma_start(
                out=xq_t,
                in_=xqT_d.rearrange("(ec p) l -> p ec l", p=128)[:, :, l0:l0 + 256])
            nc.sync.dma_start(
                out=xkv_t,
                in_=xkvT_d.rearrange("(ec p) l -> p ec l", p=128)[:, :, l0:l0 + 256])
            for t in range(2):
                ps_q = pp.tile([128, 256], f32, tag="pj", bufs=2, name="ps_q")
                for ec in range(8):
                    nc.tensor.matmul(
                        ps_q, lhsT=wq_sb[:, ec, 128 * t:128 * t + 128],
                        rhs=xq_t[:, ec, :], start=(ec == 0), stop=(ec == 7))
                nc.vector.tensor_copy(
                    out=QTS[t][g][:, 256 * half:256 * half + 256], in_=ps_q)
                ps_k = pp.tile([128, 256], f32, tag="pj", bufs=2, name="ps_k")
                for ec in range(8):
                    nc.tensor.matmul(
                        ps_k, lhsT=wk_sb[:, ec, 128 * t:128 * t + 128],
                        rhs=xkv_t[:, ec, :], start=(ec == 0), stop=(ec == 7))
                s, off = (2 * ls + 1 + 1) // 2, 0  # chunks 2ls+1, 2ls+2 -> slice ls+1
                nc.vector.tensor_copy(out=KTS[t][ls + 1], in_=ps_k)
            for sub in range(2):
                ps_v = pp.tile([128, 256], f32, tag="pj", bufs=2, name="ps_v")
                for ec in range(8):
                    nc.tensor.matmul(
                        ps_v, lhsT=xkv_t[:, ec, 128 * sub:128 * sub + 128],
                        rhs=wv_sb[:, ec, :], start=(ec == 0), stop=(ec == 7))
                nc.vector.tensor_copy(
                    out=VTS[1 + 2 * ls + sub][:, :, 0:D],
                    in_=ps_v.rearrange("p (h d) -> p h d", h=HPC))

        def attn_group(g, mts):
            gl = 512 * g
            cs = chunks[g]
            batches = [[cs[0]]] + [cs[1 + i:3 + i] for i in range(0, len(cs) - 1, 2)]
            for hp in range(HPC // 2):
                heads = (2 * hp, 2 * hp + 1)
                ctx_ps = {}
                for h in heads:
                    ctx_ps[h] = attps.tile([65, 512], f32, tag="ctx", bufs=2,
                                           name=f"ctx{h}")
                for bi, batch in enumerate(batches):
                    w = 512 * len(batch)
                    sc, pr = {}, {}
                    for h in heads:
                        sc[h] = attps.tile([128, 1024], f32, tag="sc", bufs=2,
                                           name=f"sc{h}")
                    for j, c in enumerate(batch):
                        for h in heads:
                            prow = 64 * (h % 2)
                            nc.tensor.matmul(
                                sc[h][:, 512 * j:512 * j + 512],
                                lhsT=kslice(hp, c)[prow:prow + 64, :],
                                rhs=QTS[hp][g][prow:prow + 64, :],
                                start=True, stop=True)
                    for h in heads:
                        pr[h] = attsb.tile([128, 1024], f32r, tag="probs",
                                           bufs=4, name=f"pr{h}")
                        if batch[0] == 0:
                            nc.scalar.activation(pr[h][:, 0:w], sc[h][:, 0:w],
                                                 Exp, bias=cb0[:, 0:1])
                        else:
                            nc.scalar.activation(pr[h][:, 0:w], sc[h][:, 0:w], Exp)
                    for j, c in enumerate(batch):
                        if (g, c) in mts:
                            for h in heads:
                                nc.vector.tensor_mul(
                                    pr[h][:, 512 * j:512 * j + 512],
                                    pr[h][:, 512 * j:512 * j + 512], mts[(g, c)])
                    for j, c in enumerate(batch):
                        for h in heads:
                            nc.tensor.matmul(
                                ctx_ps[h],
                                lhsT=VTS[c][:, h % 2 + 2 * hp, :],
                                rhs=pr[h][:, 512 * j:512 * j + 512],
                                start=(bi == 0 and j == 0),
                                stop=(bi == len(batches) - 1 and j == len(batch) - 1))
                for h in heads:
                    par = h % 2
                    # 1/d = exp(-ln d) on ACT: low latency, shares the exp table
                    lnd = attsb.tile([65, 512], f32, tag="lnd", bufs=2, name="lnd")
                    nc.scalar.activation(lnd[64:65, :], ctx_ps[h][64:65, :], Log)
                    rc_t = attsb.tile([65, 512], f32r, tag="recip", bufs=2, name="rc_t")
                    nc.scalar.activation(rc_t[64:65, :], lnd[64:65, :], Exp,
                                         scale=-1.0)
                    bc_ps = attps.tile([64, 512], f32, tag="sc", bufs=2, name="bc_ps")
                    nc.tensor.matmul(bc_ps, lhsT=ones_col[64:65, :],
                                     rhs=rc_t[64:65, :], start=True, stop=True)
                    bc = attsb.tile([64, 512], f32, tag="bc", bufs=2, name="bc")
                    nc.vector.tensor_copy(out=bc, in_=bc_ps)
                    if par == 0:
                        nc.vector.tensor_mul(CTXT[hp][g][0:64, :],
                                             ctx_ps[h][0:64, :], bc)
                    else:
                        st = attsb.tile([64, 512], f32r, tag="stage", bufs=2,
                                        name="st")
                        nc.vector.tensor_mul(st, ctx_ps[h][0:64, :], bc)
                        nc.sync.dma_start(out=CTXT[hp][g][64:128, :], in_=st)

        def outproj_group(g):
            gl = 512 * g
            for et in range(8):
                ops = attps.tile([128, 512], f32, tag="sc", bufs=2, name="ops")
                for hc in range(2):
                    nc.tensor.matmul(
                        ops, lhsT=wo_sb[:, hc, 128 * et:128 * et + 128],
                        rhs=CTXT[hc][g], start=(hc == 0), stop=(hc == 1))
                ot = attsb.tile([128, 512], f32, tag="ostage", bufs=3, name="ot")
                nc.vector.tensor_copy(out=ot, in_=ops)
                nc.sync.dma_start(
                    out=outT_d[128 * et:128 * et + 128, gl:gl + 512], in_=ot)

        # mask tiles preloaded up front
        mts = {}
        for g in range(NG):
            for c in chunks[g]:
                if (g, c) in mixed_idx:
                    mt = attsb.tile([128, 512], f32, tag="mask",
                                    bufs=max(nmix, 1), name=f"mt{g}_{c}")
                    nc.sync.dma_start(out=mt, in_=maskblk_d[mixed_idx[(g, c)]])
                    mts[(g, c)] = mt

        # interleaved schedule: attention g overlaps projections of later slices
        proj_slice(0)
        proj_slice(1)
        for g in range(NG):
            attn_group(g, mts)
            if g < NG - 1:
                proj_slice(2 * g + 2)
                proj_slice(2 * g + 3)
            outproj_group(g)

    nc.compile()
    return nc


def _make_plan(mask):
    """Block plan from the actual mask (union over batches -> one SPMD plan)."""
    m = np.asarray(mask[:, 0])                       # [B, LQ, LKV] bool
    blk = m.reshape(B, NG, 512, LKV // 128, 128)
    any_b = blk.any(axis=(2, 4)).any(axis=0)         # [NG, 16]
    all_b = blk.all(axis=(2, 4)).all(axis=0)         # [NG, 16]
    chunks, mixed_idx = [], {}
    order = []
    for g in range(NG):
        cl = [0]
        for c in range(1, NCH):
            if any_b[g, c - 1]:
                cl.append(c)
                if not all_b[g, c - 1]:
                    mixed_idx[(g, c)] = len(order)
                    order.append((g, c))
        chunks.append(cl)
    return {"chunks": chunks, "mixed_idx": mixed_idx, "nmix": len(order),
            "order": order}


def _prep_core_inputs(inputs, plan):
    """Per-core input dicts (8 cores: batch-major, then head-group)."""
    inputs_q = np.ascontiguousarray(inputs["inputs_q"], dtype=np.float32)
    inputs_kv = np.ascontiguousarray(inputs["inputs_kv"], dtype=np.float32)
    key_prefix = np.asarray(inputs["key_prefix"], dtype=np.float32)
    value_prefix = np.asarray(inputs["value_prefix"], dtype=np.float32)
    mask = np.asarray(inputs["mask"])
    Wq = np.asarray(inputs["Wq"], dtype=np.float32)
    Wk = np.asarray(inputs["Wk"], dtype=np.float32)
    Wv = np.asarray(inputs["Wv"], dtype=np.float32)
    Wo = np.asarray(inputs["Wo"], dtype=np.float32)

    xT = [np.ascontiguousarray(inputs_q[b].T) for b in range(B)]
    xkT = [np.ascontiguousarray(inputs_kv[b].T) for b in range(B)]

    maskblks = []
    for b in range(B):
        mb = np.empty((max(plan["nmix"], 1), 128, 512), np.float32)
        for i, (g, c) in enumerate(plan["order"]):
            mb[i] = mask[b, 0, 512 * g:512 * g + 512,
                         128 * (c - 1):128 * c].T.astype(np.float32)
        maskblks.append(mb)

    in_maps = []
    for core in range(NCORES):
        b, hg = core // HGROUPS, core % HGROUPS
        hs = slice(HPC * hg, HPC * (hg + 1))
        kpT = key_prefix[b, :, hs, :]                 # [P, 4, D]
        kpT = kpT.transpose(1, 2, 0).reshape(2, 128, P)  # [hc, (2 heads x D), P]
        kpT = np.concatenate(
            [kpT, np.zeros((2, 128, 128 - P), np.float32)], axis=2)
        kpT = np.ascontiguousarray(kpT)
        im = {
            "xqT": xT[b],
            "xkvT": xkT[b],
            "wq": np.ascontiguousarray(
                (Wq[:, hs, :] / np.sqrt(D)).reshape(E, HPC * D).astype(np.float32)),
            "wk": np.ascontiguousarray(Wk[:, hs, :].reshape(E, HPC * D)),
            "wv": np.ascontiguousarray(Wv[:, hs, :].reshape(E, HPC * D)),
            "wo": np.ascontiguousarray(Wo[hs].reshape(HPC * D, E)),
            "kprefT": kpT,
            "vpref": np.ascontiguousarray(np.concatenate(
                [value_prefix[b, :, hs, :],
                 np.zeros((128 - P, HPC, D), np.float32)], axis=0)),
            "vones": np.ones((128, NCH, HPC), np.float32),
            "onescol": np.ones((1, 64), np.float32),
        }
        if plan["nmix"]:
            im["maskblk"] = maskblks[b]
        in_maps.append(im)
    return in_maps


def kernel(**inputs) -> np.ndarray:
    from concourse import bass_utils

    plan = _make_plan(inputs["mask"])
    key = (tuple(tuple(c) for c in plan["chunks"]), tuple(plan["order"]))
    if key not in _CACHE:
        _CACHE[key] = _build_module(plan)
    nc = _CACHE[key]

    in_maps = _prep_core_inputs(inputs, plan)
    res = bass_utils.run_bass_kernel_spmd(nc, in_maps, core_ids=list(range(NCORES)))

    out = np.zeros((B, LQ, E), np.float32)
    for core in range(NCORES):
        b = core // HGROUPS
        out[b] += res.results[core]["outT"].T
    return out
